# revision 18
# baseline (speedup 1.0000x reference)
"""Trainium2 Bass kernel for a ResNet bottleneck block (training-mode BN).

Computes, for x of shape (64, 1024, 14, 14):
    y1 = relu(bn(conv1x1(x, w1)))        # 1024 -> 256
    y2 = relu(bn(conv3x3(y1, w2)))       # 256 -> 256, pad 1
    z3 = bn(conv1x1(y2, w3))             # 256 -> 1024
    out = relu(x + softplus(residual_scale) * z3)

BN is training-mode: per-channel mean/var over (N, H, W) of the full batch.
Sharding: data-parallel over the batch dim, 8 images per core; exact BN via
AllGather of per-core (sum, sumsq) + reduce on every core.

Structure (v4):
- conv1/conv2 are emitted in two passes over the contraction dim so the
  per-channel-chunk stats AllGathers overlap matmul work (passA partial is
  copied to SBUF and merged back with a fused DVE scalar_tensor_tensor).
- Per-tile stats run on the DVE right behind the merge (reduce for sums,
  square+reduce for sumsq) so the stats DMA + collective trigger fire
  mid-conv, not after an ACT accumulator drain.
- BN3 stats are computed on the PE from the Gram matrix of y2:
  sumsq(z3)[o] = w3_o^T (Y2 Y2^T) w3_o and sum(z3) = W3^T sum(y2), using a
  DMA-transposed copy of y2. The single BN3 AllGather triggers while conv3
  is still running, so the residual tail starts immediately after conv3.
- Output is written with per-image DMAs (contiguous 100KB DRAM blocks).

Conv bias is dropped (training BN is invariant to it); softplus(
residual_scale) is folded into gamma3/beta3 on the host. Matmuls run in
bf16 (fp32 PSUM). The residual x is taken from the bf16 copy of x.
"""

import os
import numpy as np
import ml_dtypes
from contextlib import ExitStack

import concourse.bass as bass
import concourse.bacc as bacc
import concourse.mybir as mybir
import concourse.tile as tile
from concourse.bass_utils import run_bass_kernel_spmd

F32 = mybir.dt.float32
BF16 = mybir.dt.bfloat16
AX = mybir.AxisListType
ALU = mybir.AluOpType
ACTF = mybir.ActivationFunctionType

N_CORES = 8
N, CIN, H, W = 64, 1024, 14, 14
P = 256
COUT = 1024
NL = N // N_CORES          # images per core (8)
HW = H * W                 # 196
F = NL * HW                # free positions per core (1568)
FT = 4                     # free-dim tiles
FTS = F // FT              # 392 positions per tile (= 2 images)
IPT = NL // FT             # images per free tile (2)
CI_CH = CIN // 128         # 8
P_CH = P // 128            # 2
CO_CH = COUT // 128        # 8
EPS = 1e-5
COUNT = N * HW             # global positions per channel (12544)
INV_N = 1.0 / COUNT
PAD = 16                   # padded spatial stride (16x16 per image)
PADF = 1664                # F padded to a multiple of 128 (13*128)
KT = PADF // 128           # transpose blocks (13)


def build():
    nc = bacc.Bacc("TRN2", target_bir_lowering=False, debug=False,
                   num_devices=N_CORES)

    # ---- I/O -------------------------------------------------------------
    xb_d = nc.dram_tensor("xb16", [CI_CH, 128, F], BF16, kind="ExternalInput")
    w1_d = nc.dram_tensor("w1t", [CI_CH, 128, P], BF16, kind="ExternalInput")
    w2_d = nc.dram_tensor("w2t", [P_CH, 128, 9, P], BF16, kind="ExternalInput")
    w3_d = nc.dram_tensor("w3t", [P_CH, 128, COUT], BF16, kind="ExternalInput")
    gb1_d = nc.dram_tensor("gb1", [2, P_CH, 128], F32, kind="ExternalInput")
    gb2_d = nc.dram_tensor("gb2", [2, P_CH, 128], F32, kind="ExternalInput")
    gb3_d = nc.dram_tensor("gb3", [2, CO_CH, 128], F32, kind="ExternalInput")
    out_d = nc.dram_tensor("out", [NL, CIN, HW], F32, kind="ExternalOutput")

    with tile.TileContext(nc) as tc, ExitStack() as ctx:
        consts = ctx.enter_context(tc.tile_pool(name="consts", bufs=1))
        xpool = ctx.enter_context(tc.tile_pool(name="xpool", bufs=1))
        actp = ctx.enter_context(tc.tile_pool(name="actp", bufs=1))
        papool = ctx.enter_context(tc.tile_pool(name="papool", bufs=1))
        statp = ctx.enter_context(tc.tile_pool(name="statp", bufs=1))
        scrp = ctx.enter_context(tc.tile_pool(name="scrp", bufs=2))
        outp = ctx.enter_context(tc.tile_pool(name="outp", bufs=4))
        dram = ctx.enter_context(tc.tile_pool(name="ccdram", bufs=1,
                                              space="DRAM"))
        psum = ctx.enter_context(tc.tile_pool(name="psum", bufs=6,
                                              space="PSUM"))
        pstat = ctx.enter_context(tc.tile_pool(name="pstat", bufs=2,
                                               space="PSUM"))

        # ---- input DMAs: x gates conv1, split across both HWDGE rings ----
        xb = [xpool.tile([128, F], BF16, name=f"xb{c}") for c in range(CI_CH)]
        # w1 first on the scalar ring (tiny, gates the first matmul)
        w1sb = consts.tile([128, CI_CH, P], BF16, name="w1sb")
        for c in range(CI_CH):
            nc.scalar.dma_start(w1sb[:, c], w1_d[c])
        for c in range(CI_CH):
            eng = nc.sync if c % 2 == 0 else nc.scalar
            eng.dma_start(xb[c][:], xb_d[c])
        w2sb = consts.tile([128, P_CH, 9, P], BF16, name="w2sb")
        for c in range(P_CH):
            nc.scalar.dma_start(w2sb[:, c], w2_d[c])
        w3sb = consts.tile([128, P_CH, COUT], BF16, name="w3sb")
        for c in range(P_CH):
            nc.scalar.dma_start(w3sb[:, c], w3_d[c])

        g1 = consts.tile([128, P_CH], F32, name="g1")
        be1 = consts.tile([128, P_CH], F32, name="be1")
        g2 = consts.tile([128, P_CH], F32, name="g2")
        be2 = consts.tile([128, P_CH], F32, name="be2")
        g3 = consts.tile([128, CO_CH], F32, name="g3")
        be3 = consts.tile([128, CO_CH], F32, name="be3")
        for t, d in ((g1, gb1_d), (g2, gb2_d), (g3, gb3_d)):
            nc.scalar.dma_start(t[:], d[0].rearrange("c p -> p c"))
        for t, d in ((be1, gb1_d), (be2, gb2_d), (be3, gb3_d)):
            nc.scalar.dma_start(t[:], d[1].rearrange("c p -> p c"))

        epst = consts.tile([128, 1], F32, name="epst")
        nc.vector.memset(epst[:], EPS)
        ones = consts.tile([128, 1], BF16, name="ones")
        nc.vector.memset(ones[:], 1.0)

        # padded bf16 activations for the 3x3 conv: [128, NL, 16, 16]
        y1p = [actp.tile([128, NL, PAD, PAD], BF16, name=f"y1p{c}")
               for c in range(P_CH)]
        for c in range(P_CH):
            nc.gpsimd.memset(y1p[c][:], 0)

        # conv passA partials
        pa = [papool.tile([128, F], BF16, name=f"pa{c}", tag=f"pa{c}")
              for c in range(4)]

        z1 = [actp.tile([128, F], F32, name=f"z1_{c}") for c in range(P_CH)]
        z2 = [actp.tile([128, F], F32, name=f"z2_{c}") for c in range(P_CH)]
        # y2 padded to PADF for the xbar transpose (pad stays zero)
        y2 = [actp.tile([128, PADF], BF16, name=f"y2_{c}") for c in range(P_CH)]
        for c in range(P_CH):
            nc.gpsimd.memset(y2[c][:, F:PADF], 0)
        z3 = [actp.tile([128, F], BF16, name=f"z3_{c}") for c in range(CO_CH)]
        y2t = actp.tile([128, KT, P], BF16, name="y2t")
        csb = actp.tile([128, P_CH, P], BF16, name="csb")
        dsb = actp.tile([128, P_CH, COUT], BF16, name="dsb")
        psb = actp.tile([128, P_CH, COUT], BF16, name="psb")
        sy = statp.tile([128, P_CH], F32, name="sy")
        syb = statp.tile([128, P_CH], BF16, name="syb")

        # ---- stats exchange helpers -------------------------------------
        def emit_stats_dma_ag(name, stc, n_ch):
            """DMA [128, 2, n_ch] stats to DRAM and AllGather them."""
            cc_mode = os.environ.get("KERNEL_CC_MODE", "ag")
            cc_in = dram.tile([2, n_ch, 128], F32, name=f"{name}_in")
            cc_out = dram.tile([N_CORES, 2, n_ch, 128], F32,
                               addr_space="Shared" if cc_mode == "ag" else "Local",
                               name=f"{name}_out")
            nc.sync.dma_start(cc_in.rearrange("s c p -> p s c"), stc[:])
            if cc_mode == "ag":
                nc.gpsimd.collective_compute(
                    "AllGather", ALU.bypass,
                    replica_groups=[list(range(N_CORES))],
                    ins=[cc_in.opt()], outs=[cc_out.opt()],
                )
            else:
                # debug: replicate local stats (wrong numerics, no cc)
                for r in range(N_CORES):
                    nc.sync.dma_start(cc_out[r], cc_in[:])
            return cc_out

        def emit_gather_params(name, cc_out, n_ch, g_ap, be_ap):
            """Pull gathered stats, reduce over cores, compute (a, b)."""
            gath = statp.tile([128, N_CORES, 2 * n_ch], F32, name=f"{name}_g")
            nc.sync.dma_start(gath[:], cc_out.rearrange("r s c p -> p r (s c)"))
            red = statp.tile([128, 2, n_ch], F32, name=f"{name}_r")
            nc.vector.tensor_reduce(
                red.rearrange("p s c -> p (s c)"),
                gath.rearrange("p r x -> p x r"), axis=AX.X, op=ALU.add)
            sums = red[:, 0, :]
            ssq = red[:, 1, :]
            s2 = statp.tile([128, n_ch], F32, name=f"{name}_s2")
            nc.vector.tensor_mul(s2[:], sums, sums)
            nv = statp.tile([128, n_ch], F32, name=f"{name}_nv")
            nc.vector.scalar_tensor_tensor(
                nv[:], s2[:], INV_N, ssq, op0=ALU.mult, op1=ALU.subtract)
            std = statp.tile([128, n_ch], F32, name=f"{name}_std")
            nc.scalar.activation(std[:], nv[:], ACTF.Sqrt, bias=epst[:, 0:1],
                                 scale=-INV_N)
            rstd = statp.tile([128, n_ch], F32, name=f"{name}_rs")
            nc.vector.reciprocal(rstd[:], std[:])
            a = statp.tile([128, n_ch], F32, name=f"{name}_a")
            nc.vector.tensor_mul(a[:], g_ap, rstd[:])
            am = statp.tile([128, n_ch], F32, name=f"{name}_am")
            nc.vector.tensor_mul(am[:], a[:], sums)
            b = statp.tile([128, n_ch], F32, name=f"{name}_b")
            nc.vector.scalar_tensor_tensor(
                b[:], am[:], -INV_N, be_ap, op0=ALU.mult, op1=ALU.add)
            return a, b

        def emit_chunk_stats(name, sp, qp, co):
            """Reduce per-tile (sum, sumsq) slots for chunk co, DMA + AG."""
            stc = statp.tile([128, 2, 1], F32, name=f"{name}_stc{co}")
            nc.vector.tensor_reduce(stc[:, 0, :], sp[:, co:co + 1, :],
                                    axis=AX.X, op=ALU.add)
            nc.vector.tensor_reduce(stc[:, 1, :], qp[:, co:co + 1, :],
                                    axis=AX.X, op=ALU.add)
            return emit_stats_dma_ag(f"{name}{co}", stc, 1)

        def emit_tile_stats(zs, sp, qp, co, ft):
            """DVE per-tile stats on the merged conv output zs."""
            nc.vector.tensor_reduce(sp[:, co, ft:ft + 1], zs, axis=AX.X,
                                    op=ALU.add)
            sq = scrp.tile([128, FTS], BF16, name="sq", tag="sq")
            nc.vector.tensor_mul(sq[:], zs, zs)
            nc.vector.tensor_reduce(qp[:, co, ft:ft + 1], sq[:], axis=AX.X,
                                    op=ALU.add)

        # ================= conv1 (1x1, 1024 -> 256), 2-pass ==============
        s1p = statp.tile([128, P_CH, FT], F32, name="s1p")
        q1p = statp.tile([128, P_CH, FT], F32, name="q1p")
        for co in range(P_CH):
            for ft in range(FT):
                pt = psum.tile([128, FTS], F32, name="pt", tag="pt")
                for ci in range(CI_CH // 2):
                    nc.tensor.matmul(
                        pt[:], w1sb[:, ci, co * 128:(co + 1) * 128],
                        xb[ci][:, ft * FTS:(ft + 1) * FTS],
                        start=(ci == 0), stop=(ci == CI_CH // 2 - 1))
                nc.scalar.activation(pa[co][:, ft * FTS:(ft + 1) * FTS],
                                     pt[:], ACTF.Copy)
        cc1 = [None, None]
        for co in range(P_CH):
            for ft in range(FT):
                fsl = slice(ft * FTS, (ft + 1) * FTS)
                pt = psum.tile([128, FTS], F32, name="pt", tag="pt")
                for i, ci in enumerate(range(CI_CH // 2, CI_CH)):
                    nc.tensor.matmul(
                        pt[:], w1sb[:, ci, co * 128:(co + 1) * 128],
                        xb[ci][:, fsl], start=(i == 0),
                        stop=(ci == CI_CH - 1))
                tb = scrp.tile([128, FTS], BF16, name="tb", tag="tb")
                nc.scalar.activation(tb[:], pt[:], ACTF.Copy)
                zs = z1[co][:, fsl]
                nc.vector.scalar_tensor_tensor(
                    zs, tb[:], 1.0, pa[co][:, fsl], op0=ALU.mult, op1=ALU.add)
                emit_tile_stats(zs, s1p, q1p, co, ft)
            cc1[co] = emit_chunk_stats("bn1", s1p, q1p, co)

        # BN1 chunk 0 -> y1p[0]; conv2 passA runs on it while AG1b flies
        a1, b1 = [None, None], [None, None]
        a1[0], b1[0] = emit_gather_params("bn1c0", cc1[0], 1,
                                          g1[:, 0:1], be1[:, 0:1])
        nc.scalar.activation(
            y1p[0][:, :, 1:1 + H, 1:1 + W],
            z1[0].rearrange("p (n h w) -> p n h w", n=NL, h=H, w=W),
            ACTF.Relu, bias=b1[0][:, 0:1], scale=a1[0][:, 0:1])

        # ================= conv2 (3x3, 256 -> 256), 2-pass ===============
        s2p = statp.tile([128, P_CH, FT], F32, name="s2p")
        q2p = statp.tile([128, P_CH, FT], F32, name="q2p")
        for co in range(P_CH):
            for ft in range(FT):
                pt = psum.tile([128, FTS], F32, name="pt", tag="pt")
                for tap in range(9):
                    ky, kx = divmod(tap, 3)
                    nc.tensor.matmul(
                        pt[:], w2sb[:, 0, tap, co * 128:(co + 1) * 128],
                        y1p[0][:, ft * IPT:(ft + 1) * IPT, ky:ky + H, kx:kx + W],
                        start=(tap == 0), stop=(tap == 8))
                nc.scalar.activation(pa[2 + co][:, ft * FTS:(ft + 1) * FTS],
                                     pt[:], ACTF.Copy)
            if co == 0:
                # BN1 chunk 1 lands mid-passA; emit its consumers here so
                # they don't head-of-line block the passA copies above
                a1[1], b1[1] = emit_gather_params("bn1c1", cc1[1], 1,
                                                  g1[:, 1:2], be1[:, 1:2])
                nc.scalar.activation(
                    y1p[1][:, :, 1:1 + H, 1:1 + W],
                    z1[1].rearrange("p (n h w) -> p n h w", n=NL, h=H, w=W),
                    ACTF.Relu, bias=b1[1][:, 0:1], scale=a1[1][:, 0:1])

        cc2 = [None, None]
        for co in range(P_CH):
            for ft in range(FT):
                fsl = slice(ft * FTS, (ft + 1) * FTS)
                pt = psum.tile([128, FTS], F32, name="pt", tag="pt")
                for tap in range(9):
                    ky, kx = divmod(tap, 3)
                    nc.tensor.matmul(
                        pt[:], w2sb[:, 1, tap, co * 128:(co + 1) * 128],
                        y1p[1][:, ft * IPT:(ft + 1) * IPT, ky:ky + H, kx:kx + W],
                        start=(tap == 0), stop=(tap == 8))
                tb = scrp.tile([128, FTS], BF16, name="tb", tag="tb")
                nc.scalar.activation(tb[:], pt[:], ACTF.Copy)
                zs = z2[co][:, fsl]
                nc.vector.scalar_tensor_tensor(
                    zs, tb[:], 1.0, pa[2 + co][:, fsl],
                    op0=ALU.mult, op1=ALU.add)
                emit_tile_stats(zs, s2p, q2p, co, ft)
            cc2[co] = emit_chunk_stats("bn2", s2p, q2p, co)

        # BN2 params + y2 applies; y2 transposes + column sums feed the
        # PE-side BN3 stats pipeline below
        a2, b2 = [None, None], [None, None]
        for c in range(P_CH):
            a2[c], b2[c] = emit_gather_params(f"bn2c{c}", cc2[c], 1,
                                              g2[:, c:c + 1], be2[:, c:c + 1])
            nc.scalar.activation(y2[c][:, :F], z2[c][:], ACTF.Relu,
                                 bias=b2[c][:, 0:1], scale=a2[c][:, 0:1])
            nc.sync.dma_start_transpose(y2t[:, :, c * 128:(c + 1) * 128],
                                        y2[c][:])
            nc.vector.tensor_reduce(sy[:, c:c + 1], y2[c][:, :F], axis=AX.X,
                                    op=ALU.add)
        nc.vector.tensor_copy(syb[:], sy[:])

        # ================= conv3 (1x1, 256 -> 1024) + BN3 stats ==========
        ptc = [None, None]
        pts = pstat.tile([128, 2, CO_CH], F32, name="pts", tag="ps")
        for co in range(CO_CH):
            for ft in range(FT):
                fsl = slice(ft * FTS, (ft + 1) * FTS)
                pt = psum.tile([128, FTS], F32, name="pt", tag="pt")
                for ci in range(P_CH):
                    nc.tensor.matmul(
                        pt[:], w3sb[:, ci, co * 128:(co + 1) * 128],
                        y2[ci][:, fsl], start=(ci == 0), stop=(ci == P_CH - 1))
                nc.scalar.activation(z3[co][:, fsl], pt[:], ACTF.Copy)
            if co == 1:
                # C = Y2 Y2^T (Gram over positions), via transposed y2
                for ic in range(P_CH):
                    ptc[ic] = pstat.tile([128, P], F32, name=f"ptc{ic}",
                                         tag="ps")
                    for k in range(KT):
                        nc.tensor.matmul(
                            ptc[ic][:], y2t[:, k, ic * 128:(ic + 1) * 128],
                            y2t[:, k, :], start=(k == 0), stop=(k == KT - 1))
                    nc.scalar.activation(csb[:, ic, :], ptc[ic][:], ACTF.Copy)
            elif co == 3:
                # D = C W3  ([256, 1024], via 4 half-width PSUM tiles)
                for jc in range(P_CH):
                    for oh in range(2):
                        ptd = pstat.tile([128, COUT // 2], F32, name="ptd",
                                         tag="ps")
                        for ic in range(P_CH):
                            nc.tensor.matmul(
                                ptd[:], csb[:, ic, jc * 128:(jc + 1) * 128],
                                w3sb[:, ic, oh * 512:(oh + 1) * 512],
                                start=(ic == 0), stop=(ic == P_CH - 1))
                        nc.scalar.activation(
                            dsb[:, jc, oh * 512:(oh + 1) * 512], ptd[:],
                            ACTF.Copy)
            elif co == 5:
                # P = W3 .* D ; sum(z3) = W3^T sum(y2)
                nc.vector.tensor_mul(psb[:], dsb[:], w3sb[:])
                for c2 in range(CO_CH):
                    for ci in range(P_CH):
                        nc.tensor.matmul(
                            pts[:, 0, c2:c2 + 1],
                            w3sb[:, ci, c2 * 128:(c2 + 1) * 128],
                            syb[:, ci:ci + 1],
                            start=(ci == 0), stop=(ci == P_CH - 1))
            elif co == 7:
                # sumsq(z3)[o] = sum_j P[j, o]
                for c2 in range(CO_CH):
                    for ci in range(P_CH):
                        nc.tensor.matmul(
                            pts[:, 1, c2:c2 + 1],
                            psb[:, ci, c2 * 128:(c2 + 1) * 128],
                            ones[:],
                            start=(ci == 0), stop=(ci == P_CH - 1))
        st3 = statp.tile([128, 2, CO_CH], F32, name="st3")
        nc.scalar.activation(st3[:], pts[:], ACTF.Copy)
        cc3 = emit_stats_dma_ag("bn3", st3, CO_CH)

        # ================= BN3 + residual tail ===========================
        a3, b3 = emit_gather_params("bn3", cc3, CO_CH, g3[:], be3[:])
        for co in range(CO_CH):
            t = scrp.tile([128, F], BF16, name="tt", tag="tt")
            nc.vector.scalar_tensor_tensor(
                t[:], z3[co][:], a3[:, co:co + 1], xb[co][:],
                op0=ALU.mult, op1=ALU.add)
            ob = outp.tile([128, F], F32, name="ob", tag="ob")
            nc.scalar.activation(ob[:], t[:], ACTF.Relu,
                                 bias=b3[:, co:co + 1])
            for n in range(NL):
                deng = nc.sync if (co * NL + n) % 2 == 0 else nc.scalar
                deng.dma_start(out_d[n, co * 128:(co + 1) * 128, :],
                               ob[:, n * HW:(n + 1) * HW])
    nc.compile()
    return nc


_NC_CACHE = None


def _get_nc():
    global _NC_CACHE
    if _NC_CACHE is None:
        _NC_CACHE = build()
    return _NC_CACHE


def _prep_host(w1, w2, w3, g1, be1, g2, be2, g3, be3, residual_scale):
    bf = ml_dtypes.bfloat16
    # conv weights, pre-transposed to [ci, ...] layouts for lhsT
    w1t = np.ascontiguousarray(
        w1.reshape(P, CIN).T.astype(bf)).reshape(CI_CH, 128, P)
    w2t = np.ascontiguousarray(
        w2.transpose(1, 2, 3, 0).astype(bf)).reshape(P_CH, 128, 9, P)
    w3t = np.ascontiguousarray(
        w3.reshape(COUT, P).T.astype(bf)).reshape(P_CH, 128, COUT)
    s = np.float32(np.log1p(np.exp(np.float64(residual_scale[0]))))
    gb1 = np.stack([g1, be1]).astype(np.float32).reshape(2, P_CH, 128)
    gb2 = np.stack([g2, be2]).astype(np.float32).reshape(2, P_CH, 128)
    gb3 = (np.stack([g3, be3]) * s).astype(np.float32).reshape(2, CO_CH, 128)
    return w1t, w2t, w3t, gb1, gb2, gb3


def prepare_in_maps(inputs):
    x = np.asarray(inputs["x"], dtype=np.float32)
    w1t, w2t, w3t, gb1, gb2, gb3 = _prep_host(
        np.asarray(inputs["w1"], np.float32), np.asarray(inputs["w2"], np.float32),
        np.asarray(inputs["w3"], np.float32), np.asarray(inputs["g1"], np.float32),
        np.asarray(inputs["be1"], np.float32), np.asarray(inputs["g2"], np.float32),
        np.asarray(inputs["be2"], np.float32), np.asarray(inputs["g3"], np.float32),
        np.asarray(inputs["be3"], np.float32),
        np.asarray(inputs["residual_scale"], np.float32),
    )
    in_maps = []
    for c in range(N_CORES):
        shard = x[c * NL:(c + 1) * NL].reshape(NL, CIN, HW)
        xb16 = np.ascontiguousarray(
            shard.transpose(1, 0, 2).astype(ml_dtypes.bfloat16)
        ).reshape(CI_CH, 128, F)
        in_maps.append({
            "xb16": xb16, "w1t": w1t, "w2t": w2t, "w3t": w3t,
            "gb1": gb1, "gb2": gb2, "gb3": gb3,
        })
    return in_maps


def kernel(**inputs):
    in_maps = prepare_in_maps(inputs)
    nc = _get_nc()
    trace = bool(int(os.environ.get("KERNEL_PROFILE", "0")))
    try:
        res = run_bass_kernel_spmd(nc, in_maps, list(range(N_CORES)), trace=trace)
    except ModuleNotFoundError:
        # axon NTFF profile hook unavailable in this container
        res = run_bass_kernel_spmd(nc, in_maps, list(range(N_CORES)), trace=False)
    if trace:
        kernel.last_exec_time_ns = getattr(res, "exec_time_ns", None)
        kernel.last_profile = res
    out = np.concatenate([res.results[c]["out"] for c in range(N_CORES)], axis=0)
    return out.reshape(N, CIN, H, W)


# revision 19
# speedup vs baseline: 1.2583x; 1.2583x over previous
"""Trainium2 Bass kernel for a ResNet bottleneck block (training-mode BN).

Computes, for x of shape (64, 1024, 14, 14):
    y1 = relu(bn(conv1x1(x, w1)))        # 1024 -> 256
    y2 = relu(bn(conv3x3(y1, w2)))       # 256 -> 256, pad 1
    z3 = bn(conv1x1(y2, w3))             # 256 -> 1024
    out = relu(x + softplus(residual_scale) * z3)

BN is training-mode: per-channel mean/var over (N, H, W) of the full batch.
Sharding: data-parallel over the batch dim, 8 images per core; exact BN via
AllGather of per-core (sum, sumsq) + reduce on every core.

Structure (v4):
- conv1/conv2 are emitted in two passes over the contraction dim so the
  per-channel-chunk stats AllGathers overlap matmul work (passA partial is
  copied to SBUF and merged back with a fused DVE scalar_tensor_tensor).
- Per-tile stats run on the DVE right behind the merge (reduce for sums,
  square+reduce for sumsq) so the stats DMA + collective trigger fire
  mid-conv, not after an ACT accumulator drain.
- BN3 stats are computed on the PE from the Gram matrix of y2:
  sumsq(z3)[o] = w3_o^T (Y2 Y2^T) w3_o and sum(z3) = W3^T sum(y2), using a
  DMA-transposed copy of y2. The single BN3 AllGather triggers while conv3
  is still running, so the residual tail starts immediately after conv3.
- Output is written with per-image DMAs (contiguous 100KB DRAM blocks).

Conv bias is dropped (training BN is invariant to it); softplus(
residual_scale) is folded into gamma3/beta3 on the host. Matmuls run in
bf16 (fp32 PSUM). The residual x is taken from the bf16 copy of x.
"""

import os
import numpy as np
import ml_dtypes
from contextlib import ExitStack

import concourse.bass as bass
import concourse.bacc as bacc
import concourse.mybir as mybir
import concourse.tile as tile
from concourse.bass_utils import run_bass_kernel_spmd

F32 = mybir.dt.float32
BF16 = mybir.dt.bfloat16
AX = mybir.AxisListType
ALU = mybir.AluOpType
ACTF = mybir.ActivationFunctionType

N_CORES = 8
N, CIN, H, W = 64, 1024, 14, 14
P = 256
COUT = 1024
NL = N // N_CORES          # images per core (8)
HW = H * W                 # 196
F = NL * HW                # free positions per core (1568)
FT = 4                     # free-dim tiles
FTS = F // FT              # 392 positions per tile (= 2 images)
IPT = NL // FT             # images per free tile (2)
CI_CH = CIN // 128         # 8
P_CH = P // 128            # 2
CO_CH = COUT // 128        # 8
EPS = 1e-5
COUNT = N * HW             # global positions per channel (12544)
INV_N = 1.0 / COUNT
PAD = 16                   # padded spatial stride (16x16 per image)
PADF = 1664                # F padded to a multiple of 128 (13*128)
KT = PADF // 128           # transpose blocks (13)


def build():
    nc = bacc.Bacc("TRN2", target_bir_lowering=False, debug=False,
                   num_devices=N_CORES)

    # ---- I/O -------------------------------------------------------------
    xb_d = nc.dram_tensor("xb16", [CI_CH, 128, F], BF16, kind="ExternalInput")
    w1_d = nc.dram_tensor("w1t", [CI_CH, 128, P], BF16, kind="ExternalInput")
    w2_d = nc.dram_tensor("w2t", [P_CH, 128, 9, P], BF16, kind="ExternalInput")
    w3_d = nc.dram_tensor("w3t", [P_CH, 128, COUT], BF16, kind="ExternalInput")
    gb1_d = nc.dram_tensor("gb1", [2, P_CH, 128], F32, kind="ExternalInput")
    gb2_d = nc.dram_tensor("gb2", [2, P_CH, 128], F32, kind="ExternalInput")
    gb3_d = nc.dram_tensor("gb3", [2, CO_CH, 128], F32, kind="ExternalInput")
    out_d = nc.dram_tensor("out", [NL, CIN, HW], F32, kind="ExternalOutput")

    with tile.TileContext(nc) as tc, ExitStack() as ctx:
        consts = ctx.enter_context(tc.tile_pool(name="consts", bufs=1))
        xpool = ctx.enter_context(tc.tile_pool(name="xpool", bufs=1))
        actp = ctx.enter_context(tc.tile_pool(name="actp", bufs=1))
        papool = ctx.enter_context(tc.tile_pool(name="papool", bufs=1))
        statp = ctx.enter_context(tc.tile_pool(name="statp", bufs=1))
        scrp = ctx.enter_context(tc.tile_pool(name="scrp", bufs=2))
        outp = ctx.enter_context(tc.tile_pool(name="outp", bufs=4))
        dram = ctx.enter_context(tc.tile_pool(name="ccdram", bufs=1,
                                              space="DRAM"))
        psum = ctx.enter_context(tc.tile_pool(name="psum", bufs=6,
                                              space="PSUM"))
        pstat = ctx.enter_context(tc.tile_pool(name="pstat", bufs=2,
                                               space="PSUM"))

        # ---- input DMAs: x gates conv1, split across both HWDGE rings ----
        xb = [xpool.tile([128, F], BF16, name=f"xb{c}") for c in range(CI_CH)]
        # w1 first on the scalar ring (tiny, gates the first matmul)
        w1sb = consts.tile([128, CI_CH, P], BF16, name="w1sb")
        for c in range(CI_CH):
            nc.scalar.dma_start(w1sb[:, c], w1_d[c])
        for c in range(CI_CH):
            eng = nc.sync if c % 2 == 0 else nc.scalar
            eng.dma_start(xb[c][:], xb_d[c])
        w2sb = consts.tile([128, P_CH, 9, P], BF16, name="w2sb")
        for c in range(P_CH):
            nc.scalar.dma_start(w2sb[:, c], w2_d[c])
        w3sb = consts.tile([128, P_CH, COUT], BF16, name="w3sb")
        for c in range(P_CH):
            nc.scalar.dma_start(w3sb[:, c], w3_d[c])

        g1 = consts.tile([128, P_CH], F32, name="g1")
        be1 = consts.tile([128, P_CH], F32, name="be1")
        g2 = consts.tile([128, P_CH], F32, name="g2")
        be2 = consts.tile([128, P_CH], F32, name="be2")
        g3 = consts.tile([128, CO_CH], F32, name="g3")
        be3 = consts.tile([128, CO_CH], F32, name="be3")
        for t, d in ((g1, gb1_d), (g2, gb2_d), (g3, gb3_d)):
            nc.scalar.dma_start(t[:], d[0].rearrange("c p -> p c"))
        for t, d in ((be1, gb1_d), (be2, gb2_d), (be3, gb3_d)):
            nc.scalar.dma_start(t[:], d[1].rearrange("c p -> p c"))

        epst = consts.tile([128, 1], F32, name="epst")
        nc.vector.memset(epst[:], EPS)
        ones = consts.tile([128, 1], BF16, name="ones")
        nc.vector.memset(ones[:], 1.0)

        # padded bf16 activations for the 3x3 conv: [128, NL, 16, 16]
        y1p = [actp.tile([128, NL, PAD, PAD], BF16, name=f"y1p{c}")
               for c in range(P_CH)]
        for c in range(P_CH):
            nc.gpsimd.memset(y1p[c][:], 0)

        # conv passA partials
        pa = [papool.tile([128, F], BF16, name=f"pa{c}", tag=f"pa{c}")
              for c in range(4)]

        z1 = [actp.tile([128, F], F32, name=f"z1_{c}") for c in range(P_CH)]
        z2 = [actp.tile([128, F], F32, name=f"z2_{c}") for c in range(P_CH)]
        # y2 padded to PADF for the xbar transpose (pad stays zero)
        y2 = [actp.tile([128, PADF], BF16, name=f"y2_{c}") for c in range(P_CH)]
        for c in range(P_CH):
            nc.gpsimd.memset(y2[c][:, F:PADF], 0)
        z3 = [actp.tile([128, F], BF16, name=f"z3_{c}") for c in range(CO_CH)]
        y2t = actp.tile([128, KT, P], BF16, name="y2t")
        csb = actp.tile([128, P_CH, P], BF16, name="csb")
        dsb = actp.tile([128, P_CH, COUT], BF16, name="dsb")
        psb = actp.tile([128, P_CH, COUT], BF16, name="psb")
        sy = statp.tile([128, P_CH], F32, name="sy")
        syb = statp.tile([128, P_CH], BF16, name="syb")

        # ---- stats exchange helpers -------------------------------------
        # Partition-major cc buffers: each partition's line is contiguous in
        # DRAM, so the in/out DMAs are descriptor-efficient. AllReduce does
        # the cross-core sum in the CCE (no gather + reduce on the core).
        def emit_stats_dma_ag(name, stc, n_ch):
            """DMA [128, 2, n_ch] stats to DRAM and AllReduce them."""
            cc_mode = os.environ.get("KERNEL_CC_MODE", "ag")
            cc_in = dram.tile([128, 2, n_ch], F32, name=f"{name}_in")
            cc_out = dram.tile([128, 2, n_ch], F32,
                               addr_space="Shared" if cc_mode == "ag" else "Local",
                               name=f"{name}_out")
            nc.sync.dma_start(cc_in[:], stc[:])
            if cc_mode == "ag":
                nc.gpsimd.collective_compute(
                    "AllReduce", ALU.add,
                    replica_groups=[list(range(N_CORES))],
                    ins=[cc_in.opt()], outs=[cc_out.opt()],
                )
            else:
                # debug: local stats only (wrong numerics, no cc)
                nc.sync.dma_start(cc_out[:], cc_in[:])
            return cc_out

        def emit_gather_params(name, cc_out, n_ch, g_ap, be_ap):
            """Pull reduced stats and compute the affine (a, b)."""
            red = statp.tile([128, 2, n_ch], F32, name=f"{name}_r")
            nc.sync.dma_start(red[:], cc_out[:])
            sums = red[:, 0, :]
            ssq = red[:, 1, :]
            s2 = statp.tile([128, n_ch], F32, name=f"{name}_s2")
            nc.vector.tensor_mul(s2[:], sums, sums)
            nv = statp.tile([128, n_ch], F32, name=f"{name}_nv")
            nc.vector.scalar_tensor_tensor(
                nv[:], s2[:], INV_N, ssq, op0=ALU.mult, op1=ALU.subtract)
            std = statp.tile([128, n_ch], F32, name=f"{name}_std")
            nc.scalar.activation(std[:], nv[:], ACTF.Sqrt, bias=epst[:, 0:1],
                                 scale=-INV_N)
            rstd = statp.tile([128, n_ch], F32, name=f"{name}_rs")
            nc.vector.reciprocal(rstd[:], std[:])
            a = statp.tile([128, n_ch], F32, name=f"{name}_a")
            nc.vector.tensor_mul(a[:], g_ap, rstd[:])
            am = statp.tile([128, n_ch], F32, name=f"{name}_am")
            nc.vector.tensor_mul(am[:], a[:], sums)
            b = statp.tile([128, n_ch], F32, name=f"{name}_b")
            nc.vector.scalar_tensor_tensor(
                b[:], am[:], -INV_N, be_ap, op0=ALU.mult, op1=ALU.add)
            return a, b

        def emit_chunk_stats(name, sp, qp, co):
            """Reduce per-tile (sum, sumsq) slots for chunk co, DMA + AG."""
            stc = statp.tile([128, 2, 1], F32, name=f"{name}_stc{co}")
            nc.vector.tensor_reduce(stc[:, 0, :], sp[:, co:co + 1, :],
                                    axis=AX.X, op=ALU.add)
            nc.vector.tensor_reduce(stc[:, 1, :], qp[:, co:co + 1, :],
                                    axis=AX.X, op=ALU.add)
            return emit_stats_dma_ag(f"{name}{co}", stc, 1)

        def emit_tile_stats(zs, sp, qp, co, ft):
            """DVE per-tile stats on the merged conv output zs."""
            nc.vector.tensor_reduce(sp[:, co, ft:ft + 1], zs, axis=AX.X,
                                    op=ALU.add)
            sq = scrp.tile([128, FTS], BF16, name="sq", tag="sq")
            nc.vector.tensor_mul(sq[:], zs, zs)
            nc.vector.tensor_reduce(qp[:, co, ft:ft + 1], sq[:], axis=AX.X,
                                    op=ALU.add)

        # ================= conv1 (1x1, 1024 -> 256), 2-pass ==============
        s1p = statp.tile([128, P_CH, FT], F32, name="s1p")
        q1p = statp.tile([128, P_CH, FT], F32, name="q1p")
        for co in range(P_CH):
            for ft in range(FT):
                pt = psum.tile([128, FTS], F32, name="pt", tag="pt")
                for ci in range(CI_CH // 2):
                    nc.tensor.matmul(
                        pt[:], w1sb[:, ci, co * 128:(co + 1) * 128],
                        xb[ci][:, ft * FTS:(ft + 1) * FTS],
                        start=(ci == 0), stop=(ci == CI_CH // 2 - 1))
                nc.scalar.activation(pa[co][:, ft * FTS:(ft + 1) * FTS],
                                     pt[:], ACTF.Copy)
        cc1 = [None, None]
        for co in range(P_CH):
            for ft in range(FT):
                fsl = slice(ft * FTS, (ft + 1) * FTS)
                pt = psum.tile([128, FTS], F32, name="pt", tag="pt")
                for i, ci in enumerate(range(CI_CH // 2, CI_CH)):
                    nc.tensor.matmul(
                        pt[:], w1sb[:, ci, co * 128:(co + 1) * 128],
                        xb[ci][:, fsl], start=(i == 0),
                        stop=(ci == CI_CH - 1))
                tb = scrp.tile([128, FTS], BF16, name="tb", tag="tb")
                nc.scalar.activation(tb[:], pt[:], ACTF.Copy)
                zs = z1[co][:, fsl]
                nc.vector.scalar_tensor_tensor(
                    zs, tb[:], 1.0, pa[co][:, fsl], op0=ALU.mult, op1=ALU.add)
                emit_tile_stats(zs, s1p, q1p, co, ft)
            cc1[co] = emit_chunk_stats("bn1", s1p, q1p, co)

        # BN1 chunk 0 -> y1p[0]; conv2 passA runs on it while AG1b flies
        a1, b1 = [None, None], [None, None]
        a1[0], b1[0] = emit_gather_params("bn1c0", cc1[0], 1,
                                          g1[:, 0:1], be1[:, 0:1])
        nc.scalar.activation(
            y1p[0][:, :, 1:1 + H, 1:1 + W],
            z1[0].rearrange("p (n h w) -> p n h w", n=NL, h=H, w=W),
            ACTF.Relu, bias=b1[0][:, 0:1], scale=a1[0][:, 0:1])

        # ================= conv2 (3x3, 256 -> 256), 2-pass ===============
        s2p = statp.tile([128, P_CH, FT], F32, name="s2p")
        q2p = statp.tile([128, P_CH, FT], F32, name="q2p")
        for co in range(P_CH):
            for ft in range(FT):
                pt = psum.tile([128, FTS], F32, name="pt", tag="pt")
                for tap in range(9):
                    ky, kx = divmod(tap, 3)
                    nc.tensor.matmul(
                        pt[:], w2sb[:, 0, tap, co * 128:(co + 1) * 128],
                        y1p[0][:, ft * IPT:(ft + 1) * IPT, ky:ky + H, kx:kx + W],
                        start=(tap == 0), stop=(tap == 8))
                nc.scalar.activation(pa[2 + co][:, ft * FTS:(ft + 1) * FTS],
                                     pt[:], ACTF.Copy)
            if co == 0:
                # BN1 chunk 1 lands mid-passA; emit its consumers here so
                # they don't head-of-line block the passA copies above
                a1[1], b1[1] = emit_gather_params("bn1c1", cc1[1], 1,
                                                  g1[:, 1:2], be1[:, 1:2])
                nc.scalar.activation(
                    y1p[1][:, :, 1:1 + H, 1:1 + W],
                    z1[1].rearrange("p (n h w) -> p n h w", n=NL, h=H, w=W),
                    ACTF.Relu, bias=b1[1][:, 0:1], scale=a1[1][:, 0:1])

        cc2 = [None, None]
        for co in range(P_CH):
            for ft in range(FT):
                fsl = slice(ft * FTS, (ft + 1) * FTS)
                pt = psum.tile([128, FTS], F32, name="pt", tag="pt")
                for tap in range(9):
                    ky, kx = divmod(tap, 3)
                    nc.tensor.matmul(
                        pt[:], w2sb[:, 1, tap, co * 128:(co + 1) * 128],
                        y1p[1][:, ft * IPT:(ft + 1) * IPT, ky:ky + H, kx:kx + W],
                        start=(tap == 0), stop=(tap == 8))
                tb = scrp.tile([128, FTS], BF16, name="tb", tag="tb")
                nc.scalar.activation(tb[:], pt[:], ACTF.Copy)
                zs = z2[co][:, fsl]
                nc.vector.scalar_tensor_tensor(
                    zs, tb[:], 1.0, pa[2 + co][:, fsl],
                    op0=ALU.mult, op1=ALU.add)
                emit_tile_stats(zs, s2p, q2p, co, ft)
            cc2[co] = emit_chunk_stats("bn2", s2p, q2p, co)

        # BN2 params + y2 applies; y2 transposes + column sums feed the
        # PE-side BN3 stats pipeline below
        a2, b2 = [None, None], [None, None]
        for c in range(P_CH):
            a2[c], b2[c] = emit_gather_params(f"bn2c{c}", cc2[c], 1,
                                              g2[:, c:c + 1], be2[:, c:c + 1])
            nc.scalar.activation(y2[c][:, :F], z2[c][:], ACTF.Relu,
                                 bias=b2[c][:, 0:1], scale=a2[c][:, 0:1])
            nc.sync.dma_start_transpose(y2t[:, :, c * 128:(c + 1) * 128],
                                        y2[c][:])
            nc.vector.tensor_reduce(sy[:, c:c + 1], y2[c][:, :F], axis=AX.X,
                                    op=ALU.add)
        nc.vector.tensor_copy(syb[:], sy[:])

        # ================= conv3 (1x1, 256 -> 1024) + BN3 stats ==========
        ptc = [None, None]
        pts = pstat.tile([128, 2, CO_CH], F32, name="pts", tag="ps")
        for co in range(CO_CH):
            for ft in range(FT):
                fsl = slice(ft * FTS, (ft + 1) * FTS)
                pt = psum.tile([128, FTS], F32, name="pt", tag="pt")
                for ci in range(P_CH):
                    nc.tensor.matmul(
                        pt[:], w3sb[:, ci, co * 128:(co + 1) * 128],
                        y2[ci][:, fsl], start=(ci == 0), stop=(ci == P_CH - 1))
                nc.scalar.activation(z3[co][:, fsl], pt[:], ACTF.Copy)
            if co == 1:
                # Full BN3 stats pipeline, emitted early so the AllReduce
                # overlaps the rest of conv3.
                # C = Y2 Y2^T (Gram over positions), via transposed y2
                for ic in range(P_CH):
                    ptc[ic] = pstat.tile([128, P], F32, name=f"ptc{ic}",
                                         tag="ps")
                    for k in range(KT):
                        nc.tensor.matmul(
                            ptc[ic][:], y2t[:, k, ic * 128:(ic + 1) * 128],
                            y2t[:, k, :], start=(k == 0), stop=(k == KT - 1))
                    nc.scalar.activation(csb[:, ic, :], ptc[ic][:], ACTF.Copy)
                # D = C W3  ([256, 1024], via 4 half-width PSUM tiles)
                for jc in range(P_CH):
                    for oh in range(2):
                        ptd = pstat.tile([128, COUT // 2], F32, name="ptd",
                                         tag="ps")
                        for ic in range(P_CH):
                            nc.tensor.matmul(
                                ptd[:], csb[:, ic, jc * 128:(jc + 1) * 128],
                                w3sb[:, ic, oh * 512:(oh + 1) * 512],
                                start=(ic == 0), stop=(ic == P_CH - 1))
                        nc.scalar.activation(
                            dsb[:, jc, oh * 512:(oh + 1) * 512], ptd[:],
                            ACTF.Copy)
                # P = W3 .* D ; sum(z3) = W3^T sum(y2)
                nc.vector.tensor_mul(psb[:], dsb[:], w3sb[:])
                for c2 in range(CO_CH):
                    for ci in range(P_CH):
                        nc.tensor.matmul(
                            pts[:, 0, c2:c2 + 1],
                            w3sb[:, ci, c2 * 128:(c2 + 1) * 128],
                            syb[:, ci:ci + 1],
                            start=(ci == 0), stop=(ci == P_CH - 1))
                # sumsq(z3)[o] = sum_j P[j, o]
                for c2 in range(CO_CH):
                    for ci in range(P_CH):
                        nc.tensor.matmul(
                            pts[:, 1, c2:c2 + 1],
                            psb[:, ci, c2 * 128:(c2 + 1) * 128],
                            ones[:],
                            start=(ci == 0), stop=(ci == P_CH - 1))
                st3 = statp.tile([128, 2, CO_CH], F32, name="st3")
                nc.scalar.activation(st3[:], pts[:], ACTF.Copy)
                cc3 = emit_stats_dma_ag("bn3", st3, CO_CH)

        # ================= BN3 + residual tail ===========================
        a3, b3 = emit_gather_params("bn3", cc3, CO_CH, g3[:], be3[:])
        for co in range(CO_CH):
            t = scrp.tile([128, F], BF16, name="tt", tag="tt")
            nc.vector.scalar_tensor_tensor(
                t[:], z3[co][:], a3[:, co:co + 1], xb[co][:],
                op0=ALU.mult, op1=ALU.add)
            ob = outp.tile([128, F], F32, name="ob", tag="ob")
            nc.scalar.activation(ob[:], t[:], ACTF.Relu,
                                 bias=b3[:, co:co + 1])
            for n in range(NL):
                deng = nc.sync if (co * NL + n) % 2 == 0 else nc.scalar
                deng.dma_start(out_d[n, co * 128:(co + 1) * 128, :],
                               ob[:, n * HW:(n + 1) * HW])
    nc.compile()
    return nc


_NC_CACHE = None


def _get_nc():
    global _NC_CACHE
    if _NC_CACHE is None:
        _NC_CACHE = build()
    return _NC_CACHE


def _prep_host(w1, w2, w3, g1, be1, g2, be2, g3, be3, residual_scale):
    bf = ml_dtypes.bfloat16
    # conv weights, pre-transposed to [ci, ...] layouts for lhsT
    w1t = np.ascontiguousarray(
        w1.reshape(P, CIN).T.astype(bf)).reshape(CI_CH, 128, P)
    w2t = np.ascontiguousarray(
        w2.transpose(1, 2, 3, 0).astype(bf)).reshape(P_CH, 128, 9, P)
    w3t = np.ascontiguousarray(
        w3.reshape(COUT, P).T.astype(bf)).reshape(P_CH, 128, COUT)
    s = np.float32(np.log1p(np.exp(np.float64(residual_scale[0]))))
    gb1 = np.stack([g1, be1]).astype(np.float32).reshape(2, P_CH, 128)
    gb2 = np.stack([g2, be2]).astype(np.float32).reshape(2, P_CH, 128)
    gb3 = (np.stack([g3, be3]) * s).astype(np.float32).reshape(2, CO_CH, 128)
    return w1t, w2t, w3t, gb1, gb2, gb3


def prepare_in_maps(inputs):
    x = np.asarray(inputs["x"], dtype=np.float32)
    w1t, w2t, w3t, gb1, gb2, gb3 = _prep_host(
        np.asarray(inputs["w1"], np.float32), np.asarray(inputs["w2"], np.float32),
        np.asarray(inputs["w3"], np.float32), np.asarray(inputs["g1"], np.float32),
        np.asarray(inputs["be1"], np.float32), np.asarray(inputs["g2"], np.float32),
        np.asarray(inputs["be2"], np.float32), np.asarray(inputs["g3"], np.float32),
        np.asarray(inputs["be3"], np.float32),
        np.asarray(inputs["residual_scale"], np.float32),
    )
    in_maps = []
    for c in range(N_CORES):
        shard = x[c * NL:(c + 1) * NL].reshape(NL, CIN, HW)
        xb16 = np.ascontiguousarray(
            shard.transpose(1, 0, 2).astype(ml_dtypes.bfloat16)
        ).reshape(CI_CH, 128, F)
        in_maps.append({
            "xb16": xb16, "w1t": w1t, "w2t": w2t, "w3t": w3t,
            "gb1": gb1, "gb2": gb2, "gb3": gb3,
        })
    return in_maps


def kernel(**inputs):
    in_maps = prepare_in_maps(inputs)
    nc = _get_nc()
    trace = bool(int(os.environ.get("KERNEL_PROFILE", "0")))
    try:
        res = run_bass_kernel_spmd(nc, in_maps, list(range(N_CORES)), trace=trace)
    except ModuleNotFoundError:
        # axon NTFF profile hook unavailable in this container
        res = run_bass_kernel_spmd(nc, in_maps, list(range(N_CORES)), trace=False)
    if trace:
        kernel.last_exec_time_ns = getattr(res, "exec_time_ns", None)
        kernel.last_profile = res
    out = np.concatenate([res.results[c]["out"] for c in range(N_CORES)], axis=0)
    return out.reshape(N, CIN, H, W)


# revision 20
# speedup vs baseline: 1.3560x; 1.0777x over previous
"""Trainium2 Bass kernel for a ResNet bottleneck block (training-mode BN).

Computes, for x of shape (64, 1024, 14, 14):
    y1 = relu(bn(conv1x1(x, w1)))        # 1024 -> 256
    y2 = relu(bn(conv3x3(y1, w2)))       # 256 -> 256, pad 1
    z3 = bn(conv1x1(y2, w3))             # 256 -> 1024
    out = relu(x + softplus(residual_scale) * z3)

BN is training-mode: per-channel mean/var over (N, H, W) of the full batch.
Sharding: data-parallel over the batch dim, 8 images per core; exact BN via
AllGather of per-core (sum, sumsq) + reduce on every core.

Structure (v4):
- conv1/conv2 are emitted in two passes over the contraction dim so the
  per-channel-chunk stats AllGathers overlap matmul work (passA partial is
  copied to SBUF and merged back with a fused DVE scalar_tensor_tensor).
- Per-tile stats run on the DVE right behind the merge (reduce for sums,
  square+reduce for sumsq) so the stats DMA + collective trigger fire
  mid-conv, not after an ACT accumulator drain.
- BN3 stats are computed on the PE from the Gram matrix of y2:
  sumsq(z3)[o] = w3_o^T (Y2 Y2^T) w3_o and sum(z3) = W3^T sum(y2), using a
  DMA-transposed copy of y2. The single BN3 AllGather triggers while conv3
  is still running, so the residual tail starts immediately after conv3.
- Output is written with per-image DMAs (contiguous 100KB DRAM blocks).

Conv bias is dropped (training BN is invariant to it); softplus(
residual_scale) is folded into gamma3/beta3 on the host. Matmuls run in
bf16 (fp32 PSUM). The residual x is taken from the bf16 copy of x.
"""

import os
import numpy as np
import ml_dtypes
from contextlib import ExitStack

import concourse.bass as bass
import concourse.bacc as bacc
import concourse.mybir as mybir
import concourse.tile as tile
from concourse.bass_utils import run_bass_kernel_spmd

F32 = mybir.dt.float32
BF16 = mybir.dt.bfloat16
AX = mybir.AxisListType
ALU = mybir.AluOpType
ACTF = mybir.ActivationFunctionType

N_CORES = 8
N, CIN, H, W = 64, 1024, 14, 14
P = 256
COUT = 1024
NL = N // N_CORES          # images per core (8)
HW = H * W                 # 196
F = NL * HW                # free positions per core (1568)
FT = 4                     # free-dim tiles
FTS = F // FT              # 392 positions per tile (= 2 images)
IPT = NL // FT             # images per free tile (2)
CI_CH = CIN // 128         # 8
P_CH = P // 128            # 2
CO_CH = COUT // 128        # 8
EPS = 1e-5
COUNT = N * HW             # global positions per channel (12544)
INV_N = 1.0 / COUNT
PAD = 16                   # padded spatial stride (16x16 per image)
PADF = 1664                # F padded to a multiple of 128 (13*128)
KT = PADF // 128           # transpose blocks (13)


def build():
    nc = bacc.Bacc("TRN2", target_bir_lowering=False, debug=False,
                   num_devices=N_CORES)

    # ---- I/O -------------------------------------------------------------
    xb_d = nc.dram_tensor("xb16", [CI_CH, 128, F], BF16, kind="ExternalInput")
    w1_d = nc.dram_tensor("w1t", [CI_CH, 128, P], BF16, kind="ExternalInput")
    w2_d = nc.dram_tensor("w2t", [P_CH, 128, 9, P], BF16, kind="ExternalInput")
    w3_d = nc.dram_tensor("w3t", [P_CH, 128, COUT], BF16, kind="ExternalInput")
    gb1_d = nc.dram_tensor("gb1", [2, P_CH, 128], F32, kind="ExternalInput")
    gb2_d = nc.dram_tensor("gb2", [2, P_CH, 128], F32, kind="ExternalInput")
    gb3_d = nc.dram_tensor("gb3", [2, CO_CH, 128], F32, kind="ExternalInput")
    out_d = nc.dram_tensor("out", [NL, CIN, HW], F32, kind="ExternalOutput")

    with tile.TileContext(nc) as tc, ExitStack() as ctx:
        consts = ctx.enter_context(tc.tile_pool(name="consts", bufs=1))
        xpool = ctx.enter_context(tc.tile_pool(name="xpool", bufs=1))
        actp = ctx.enter_context(tc.tile_pool(name="actp", bufs=1))
        papool = ctx.enter_context(tc.tile_pool(name="papool", bufs=1))
        statp = ctx.enter_context(tc.tile_pool(name="statp", bufs=1))
        scrp = ctx.enter_context(tc.tile_pool(name="scrp", bufs=2))
        outp = ctx.enter_context(tc.tile_pool(name="outp", bufs=4))
        dram = ctx.enter_context(tc.tile_pool(name="ccdram", bufs=1,
                                              space="DRAM"))
        psum = ctx.enter_context(tc.tile_pool(name="psum", bufs=6,
                                              space="PSUM"))
        pstat = ctx.enter_context(tc.tile_pool(name="pstat", bufs=2,
                                               space="PSUM"))

        # ---- input DMAs: x gates conv1, split across both HWDGE rings ----
        xb = [xpool.tile([128, F], BF16, name=f"xb{c}") for c in range(CI_CH)]
        # w1 first on the scalar ring (tiny, gates the first matmul)
        w1sb = consts.tile([128, CI_CH, P], BF16, name="w1sb")
        for c in range(CI_CH):
            nc.scalar.dma_start(w1sb[:, c], w1_d[c])
        for c in range(CI_CH):
            eng = nc.sync if c % 2 == 0 else nc.scalar
            eng.dma_start(xb[c][:], xb_d[c])
        w2sb = consts.tile([128, P_CH, 9, P], BF16, name="w2sb")
        for c in range(P_CH):
            nc.scalar.dma_start(w2sb[:, c], w2_d[c])
        w3sb = consts.tile([128, P_CH, COUT], BF16, name="w3sb")
        for c in range(P_CH):
            nc.scalar.dma_start(w3sb[:, c], w3_d[c])

        g1 = consts.tile([128, P_CH], F32, name="g1")
        be1 = consts.tile([128, P_CH], F32, name="be1")
        g2 = consts.tile([128, P_CH], F32, name="g2")
        be2 = consts.tile([128, P_CH], F32, name="be2")
        g3 = consts.tile([128, CO_CH], F32, name="g3")
        be3 = consts.tile([128, CO_CH], F32, name="be3")
        for t, d in ((g1, gb1_d), (g2, gb2_d), (g3, gb3_d)):
            nc.scalar.dma_start(t[:], d[0].rearrange("c p -> p c"))
        for t, d in ((be1, gb1_d), (be2, gb2_d), (be3, gb3_d)):
            nc.scalar.dma_start(t[:], d[1].rearrange("c p -> p c"))

        epst = consts.tile([128, 1], F32, name="epst")
        nc.vector.memset(epst[:], EPS)
        ones = consts.tile([128, 1], BF16, name="ones")
        nc.vector.memset(ones[:], 1.0)

        # padded bf16 activations for the 3x3 conv: [128, NL, 16, 16]
        y1p = [actp.tile([128, NL, PAD, PAD], BF16, name=f"y1p{c}")
               for c in range(P_CH)]
        for c in range(P_CH):
            nc.gpsimd.memset(y1p[c][:], 0)

        # conv passA partials
        pa = [papool.tile([128, F], BF16, name=f"pa{c}", tag=f"pa{c}")
              for c in range(4)]

        z1 = [actp.tile([128, F], F32, name=f"z1_{c}") for c in range(P_CH)]
        z2 = [actp.tile([128, F], F32, name=f"z2_{c}") for c in range(P_CH)]
        # y2 padded to PADF for the xbar transpose (pad stays zero)
        y2 = [actp.tile([128, PADF], BF16, name=f"y2_{c}") for c in range(P_CH)]
        for c in range(P_CH):
            nc.gpsimd.memset(y2[c][:, F:PADF], 0)
        z3 = [actp.tile([128, F], BF16, name=f"z3_{c}") for c in range(CO_CH)]
        y2t = actp.tile([128, KT, P], BF16, name="y2t")
        csb = actp.tile([128, P_CH, P], BF16, name="csb")
        dsb = actp.tile([128, P_CH, COUT], BF16, name="dsb")
        psb = actp.tile([128, P_CH, COUT], BF16, name="psb")
        sy = statp.tile([128, P_CH], F32, name="sy")
        syb = statp.tile([128, P_CH], BF16, name="syb")

        # ---- stats exchange helpers -------------------------------------
        # Partition-major cc buffers: each partition's line is contiguous in
        # DRAM, so the in/out DMAs are descriptor-efficient. AllReduce does
        # the cross-core sum in the CCE (no gather + reduce on the core).
        def emit_stats_dma_ag(name, stc, n_ch):
            """DMA [128, 2, n_ch] stats to DRAM and AllReduce them."""
            cc_mode = os.environ.get("KERNEL_CC_MODE", "ag")
            cc_in = dram.tile([128, 2, n_ch], F32, name=f"{name}_in")
            cc_out = dram.tile([128, 2, n_ch], F32,
                               addr_space="Shared" if cc_mode == "ag" else "Local",
                               name=f"{name}_out")
            nc.sync.dma_start(cc_in[:], stc[:])
            if cc_mode == "ag":
                nc.gpsimd.collective_compute(
                    "AllReduce", ALU.add,
                    replica_groups=[list(range(N_CORES))],
                    ins=[cc_in.opt()], outs=[cc_out.opt()],
                )
            else:
                # debug: local stats only (wrong numerics, no cc)
                nc.sync.dma_start(cc_out[:], cc_in[:])
            return cc_out

        def emit_gather_params(name, cc_out, n_ch, g_ap, be_ap):
            """Pull reduced stats and compute the affine (a, b)."""
            red = statp.tile([128, 2, n_ch], F32, name=f"{name}_r")
            nc.sync.dma_start(red[:], cc_out[:])
            sums = red[:, 0, :]
            ssq = red[:, 1, :]
            s2 = statp.tile([128, n_ch], F32, name=f"{name}_s2")
            nc.vector.tensor_mul(s2[:], sums, sums)
            nv = statp.tile([128, n_ch], F32, name=f"{name}_nv")
            nc.vector.scalar_tensor_tensor(
                nv[:], s2[:], INV_N, ssq, op0=ALU.mult, op1=ALU.subtract)
            std = statp.tile([128, n_ch], F32, name=f"{name}_std")
            nc.scalar.activation(std[:], nv[:], ACTF.Sqrt, bias=epst[:, 0:1],
                                 scale=-INV_N)
            rstd = statp.tile([128, n_ch], F32, name=f"{name}_rs")
            nc.vector.reciprocal(rstd[:], std[:])
            a = statp.tile([128, n_ch], F32, name=f"{name}_a")
            nc.vector.tensor_mul(a[:], g_ap, rstd[:])
            am = statp.tile([128, n_ch], F32, name=f"{name}_am")
            nc.vector.tensor_mul(am[:], a[:], sums)
            b = statp.tile([128, n_ch], F32, name=f"{name}_b")
            nc.vector.scalar_tensor_tensor(
                b[:], am[:], -INV_N, be_ap, op0=ALU.mult, op1=ALU.add)
            return a, b

        def emit_sumsq(name, b6, chunks):
            """bn_aggr per chunk + convert (mean, var) -> (sum, sumsq)."""
            nch = len(chunks)
            mv = statp.tile([128, nch, 2], F32, name=f"{name}_mv")
            for i, co in enumerate(chunks):
                nc.vector.bn_aggr(mv[:, i, :], b6[:, co, :, :])
            stc = statp.tile([128, 2, nch], F32, name=f"{name}_stc")
            nc.vector.tensor_scalar_mul(stc[:, 0, :], mv[:, :, 0], float(F))
            m2 = statp.tile([128, nch], F32, name=f"{name}_m2")
            nc.vector.tensor_mul(m2[:], mv[:, :, 0], mv[:, :, 0])
            t2 = statp.tile([128, nch], F32, name=f"{name}_t2")
            nc.vector.tensor_tensor(t2[:], m2[:], mv[:, :, 1], ALU.add)
            nc.vector.tensor_scalar_mul(stc[:, 1, :], t2[:], float(F))
            return emit_stats_dma_ag(name, stc, nch)

        # ================= conv1 (1x1, 1024 -> 256), 2-pass ==============
        b61 = statp.tile([128, P_CH, FT, 6], F32, name="b61")
        for co in range(P_CH):
            for ft in range(FT):
                pt = psum.tile([128, FTS], F32, name="pt", tag="pt")
                for ci in range(CI_CH // 2):
                    nc.tensor.matmul(
                        pt[:], w1sb[:, ci, co * 128:(co + 1) * 128],
                        xb[ci][:, ft * FTS:(ft + 1) * FTS],
                        start=(ci == 0), stop=(ci == CI_CH // 2 - 1))
                nc.scalar.activation(pa[co][:, ft * FTS:(ft + 1) * FTS],
                                     pt[:], ACTF.Copy)
        cc1 = [None, None]
        for co in range(P_CH):
            for ft in range(FT):
                fsl = slice(ft * FTS, (ft + 1) * FTS)
                pt = psum.tile([128, FTS], F32, name="pt", tag="pt")
                for i, ci in enumerate(range(CI_CH // 2, CI_CH)):
                    nc.tensor.matmul(
                        pt[:], w1sb[:, ci, co * 128:(co + 1) * 128],
                        xb[ci][:, fsl], start=(i == 0),
                        stop=(ci == CI_CH - 1))
                tb = scrp.tile([128, FTS], BF16, name="tb", tag="tb")
                nc.scalar.activation(tb[:], pt[:], ACTF.Copy)
                zs = z1[co][:, fsl]
                nc.vector.scalar_tensor_tensor(
                    zs, tb[:], 1.0, pa[co][:, fsl], op0=ALU.mult, op1=ALU.add)
                nc.vector.bn_stats(b61[:, co, ft, :], zs)
            cc1[co] = emit_sumsq(f"bn1{co}", b61, [co])

        # BN1 chunk 0 -> y1p[0]; conv2 passA runs on it while AG1b flies
        a1, b1 = [None, None], [None, None]
        a1[0], b1[0] = emit_gather_params("bn1c0", cc1[0], 1,
                                          g1[:, 0:1], be1[:, 0:1])
        nc.scalar.activation(
            y1p[0][:, :, 1:1 + H, 1:1 + W],
            z1[0].rearrange("p (n h w) -> p n h w", n=NL, h=H, w=W),
            ACTF.Relu, bias=b1[0][:, 0:1], scale=a1[0][:, 0:1])

        # ================= conv2 (3x3, 256 -> 256), 2-pass ===============
        b62 = statp.tile([128, P_CH, FT, 6], F32, name="b62")
        for co in range(P_CH):
            for ft in range(FT):
                pt = psum.tile([128, FTS], F32, name="pt", tag="pt")
                for tap in range(9):
                    ky, kx = divmod(tap, 3)
                    nc.tensor.matmul(
                        pt[:], w2sb[:, 0, tap, co * 128:(co + 1) * 128],
                        y1p[0][:, ft * IPT:(ft + 1) * IPT, ky:ky + H, kx:kx + W],
                        start=(tap == 0), stop=(tap == 8))
                nc.scalar.activation(pa[2 + co][:, ft * FTS:(ft + 1) * FTS],
                                     pt[:], ACTF.Copy)
            if co == 0:
                # BN1 chunk 1 lands mid-passA; emit its consumers here so
                # they don't head-of-line block the passA copies above
                a1[1], b1[1] = emit_gather_params("bn1c1", cc1[1], 1,
                                                  g1[:, 1:2], be1[:, 1:2])
                nc.scalar.activation(
                    y1p[1][:, :, 1:1 + H, 1:1 + W],
                    z1[1].rearrange("p (n h w) -> p n h w", n=NL, h=H, w=W),
                    ACTF.Relu, bias=b1[1][:, 0:1], scale=a1[1][:, 0:1])

        for co in range(P_CH):
            for ft in range(FT):
                fsl = slice(ft * FTS, (ft + 1) * FTS)
                pt = psum.tile([128, FTS], F32, name="pt", tag="pt")
                for tap in range(9):
                    ky, kx = divmod(tap, 3)
                    nc.tensor.matmul(
                        pt[:], w2sb[:, 1, tap, co * 128:(co + 1) * 128],
                        y1p[1][:, ft * IPT:(ft + 1) * IPT, ky:ky + H, kx:kx + W],
                        start=(tap == 0), stop=(tap == 8))
                tb = scrp.tile([128, FTS], BF16, name="tb", tag="tb")
                nc.scalar.activation(tb[:], pt[:], ACTF.Copy)
                zs = z2[co][:, fsl]
                nc.vector.scalar_tensor_tensor(
                    zs, tb[:], 1.0, pa[2 + co][:, fsl],
                    op0=ALU.mult, op1=ALU.add)
                nc.vector.bn_stats(b62[:, co, ft, :], zs)
        # single merged BN2 exchange (conv3 needs both chunks anyway)
        cc2 = emit_sumsq("bn2", b62, [0, 1])

        # BN2 params + y2 applies; y2 transposes + column sums feed the
        # PE-side BN3 stats pipeline below
        a2, b2 = emit_gather_params("bn2", cc2, P_CH, g2[:], be2[:])
        for c in range(P_CH):
            nc.scalar.activation(y2[c][:, :F], z2[c][:], ACTF.Relu,
                                 bias=b2[:, c:c + 1], scale=a2[:, c:c + 1])
            nc.sync.dma_start_transpose(y2t[:, :, c * 128:(c + 1) * 128],
                                        y2[c][:])
            nc.vector.tensor_reduce(sy[:, c:c + 1], y2[c][:, :F], axis=AX.X,
                                    op=ALU.add)
        nc.vector.tensor_copy(syb[:], sy[:])

        # ================= conv3 (1x1, 256 -> 1024) + BN3 stats ==========
        ptc = [None, None]
        pts = pstat.tile([128, 2, CO_CH], F32, name="pts", tag="ps")
        for co in range(CO_CH):
            for ft in range(FT):
                fsl = slice(ft * FTS, (ft + 1) * FTS)
                pt = psum.tile([128, FTS], F32, name="pt", tag="pt")
                for ci in range(P_CH):
                    nc.tensor.matmul(
                        pt[:], w3sb[:, ci, co * 128:(co + 1) * 128],
                        y2[ci][:, fsl], start=(ci == 0), stop=(ci == P_CH - 1))
                nc.scalar.activation(z3[co][:, fsl], pt[:], ACTF.Copy)
            if co == 1:
                # Full BN3 stats pipeline, emitted early so the AllReduce
                # overlaps the rest of conv3.
                # C = Y2 Y2^T (Gram over positions), via transposed y2
                for ic in range(P_CH):
                    ptc[ic] = pstat.tile([128, P], F32, name=f"ptc{ic}",
                                         tag="ps")
                    for k in range(KT):
                        nc.tensor.matmul(
                            ptc[ic][:], y2t[:, k, ic * 128:(ic + 1) * 128],
                            y2t[:, k, :], start=(k == 0), stop=(k == KT - 1))
                    nc.scalar.activation(csb[:, ic, :], ptc[ic][:], ACTF.Copy)
                # D = C W3  ([256, 1024], via 4 half-width PSUM tiles)
                for jc in range(P_CH):
                    for oh in range(2):
                        ptd = pstat.tile([128, COUT // 2], F32, name="ptd",
                                         tag="ps")
                        for ic in range(P_CH):
                            nc.tensor.matmul(
                                ptd[:], csb[:, ic, jc * 128:(jc + 1) * 128],
                                w3sb[:, ic, oh * 512:(oh + 1) * 512],
                                start=(ic == 0), stop=(ic == P_CH - 1))
                        nc.scalar.activation(
                            dsb[:, jc, oh * 512:(oh + 1) * 512], ptd[:],
                            ACTF.Copy)
                # P = W3 .* D ; sum(z3) = W3^T sum(y2)
                nc.vector.tensor_mul(psb[:], dsb[:], w3sb[:])
                for c2 in range(CO_CH):
                    for ci in range(P_CH):
                        nc.tensor.matmul(
                            pts[:, 0, c2:c2 + 1],
                            w3sb[:, ci, c2 * 128:(c2 + 1) * 128],
                            syb[:, ci:ci + 1],
                            start=(ci == 0), stop=(ci == P_CH - 1))
                # sumsq(z3)[o] = sum_j P[j, o]
                for c2 in range(CO_CH):
                    for ci in range(P_CH):
                        nc.tensor.matmul(
                            pts[:, 1, c2:c2 + 1],
                            psb[:, ci, c2 * 128:(c2 + 1) * 128],
                            ones[:],
                            start=(ci == 0), stop=(ci == P_CH - 1))
                st3 = statp.tile([128, 2, CO_CH], F32, name="st3")
                nc.scalar.activation(st3[:], pts[:], ACTF.Copy)
                cc3 = emit_stats_dma_ag("bn3", st3, CO_CH)

        # ================= BN3 + residual tail ===========================
        a3, b3 = emit_gather_params("bn3", cc3, CO_CH, g3[:], be3[:])
        for co in range(CO_CH):
            t = scrp.tile([128, F], BF16, name="tt", tag="tt")
            nc.vector.scalar_tensor_tensor(
                t[:], z3[co][:], a3[:, co:co + 1], xb[co][:],
                op0=ALU.mult, op1=ALU.add)
            ob = outp.tile([128, F], F32, name="ob", tag="ob")
            nc.scalar.activation(ob[:], t[:], ACTF.Relu,
                                 bias=b3[:, co:co + 1])
            deng = nc.sync if co % 2 == 0 else nc.scalar
            deng.dma_start(
                out_d[:, co * 128:(co + 1) * 128, :].rearrange(
                    "n p f -> p n f"), ob[:])
    nc.compile()
    return nc


_NC_CACHE = None


def _get_nc():
    global _NC_CACHE
    if _NC_CACHE is None:
        _NC_CACHE = build()
    return _NC_CACHE


def _prep_host(w1, w2, w3, g1, be1, g2, be2, g3, be3, residual_scale):
    bf = ml_dtypes.bfloat16
    # conv weights, pre-transposed to [ci, ...] layouts for lhsT
    w1t = np.ascontiguousarray(
        w1.reshape(P, CIN).T.astype(bf)).reshape(CI_CH, 128, P)
    w2t = np.ascontiguousarray(
        w2.transpose(1, 2, 3, 0).astype(bf)).reshape(P_CH, 128, 9, P)
    w3t = np.ascontiguousarray(
        w3.reshape(COUT, P).T.astype(bf)).reshape(P_CH, 128, COUT)
    s = np.float32(np.log1p(np.exp(np.float64(residual_scale[0]))))
    gb1 = np.stack([g1, be1]).astype(np.float32).reshape(2, P_CH, 128)
    gb2 = np.stack([g2, be2]).astype(np.float32).reshape(2, P_CH, 128)
    gb3 = (np.stack([g3, be3]) * s).astype(np.float32).reshape(2, CO_CH, 128)
    return w1t, w2t, w3t, gb1, gb2, gb3


def prepare_in_maps(inputs):
    x = np.asarray(inputs["x"], dtype=np.float32)
    w1t, w2t, w3t, gb1, gb2, gb3 = _prep_host(
        np.asarray(inputs["w1"], np.float32), np.asarray(inputs["w2"], np.float32),
        np.asarray(inputs["w3"], np.float32), np.asarray(inputs["g1"], np.float32),
        np.asarray(inputs["be1"], np.float32), np.asarray(inputs["g2"], np.float32),
        np.asarray(inputs["be2"], np.float32), np.asarray(inputs["g3"], np.float32),
        np.asarray(inputs["be3"], np.float32),
        np.asarray(inputs["residual_scale"], np.float32),
    )
    in_maps = []
    for c in range(N_CORES):
        shard = x[c * NL:(c + 1) * NL].reshape(NL, CIN, HW)
        xb16 = np.ascontiguousarray(
            shard.transpose(1, 0, 2).astype(ml_dtypes.bfloat16)
        ).reshape(CI_CH, 128, F)
        in_maps.append({
            "xb16": xb16, "w1t": w1t, "w2t": w2t, "w3t": w3t,
            "gb1": gb1, "gb2": gb2, "gb3": gb3,
        })
    return in_maps


def kernel(**inputs):
    in_maps = prepare_in_maps(inputs)
    nc = _get_nc()
    trace = bool(int(os.environ.get("KERNEL_PROFILE", "0")))
    try:
        res = run_bass_kernel_spmd(nc, in_maps, list(range(N_CORES)), trace=trace)
    except ModuleNotFoundError:
        # axon NTFF profile hook unavailable in this container
        res = run_bass_kernel_spmd(nc, in_maps, list(range(N_CORES)), trace=False)
    if trace:
        kernel.last_exec_time_ns = getattr(res, "exec_time_ns", None)
        kernel.last_profile = res
    out = np.concatenate([res.results[c]["out"] for c in range(N_CORES)], axis=0)
    return out.reshape(N, CIN, H, W)
